# revision 10
# baseline (speedup 1.0000x reference)
"""Binarized 3x3 conv (stride 1, pad 1) + bias on 8 Trainium2 NeuronCores.

Full problem: x[32,256,56,56] f32, weight[256,256,3,3] f32, bias[256] f32
-> y[32,256,56,56] f32 with y = conv2d(sign(x), sign(weight), pad=1) + bias
(sign(t) = +1 for t >= 0 else -1).

Sharding: data-parallel over batch. Each of the 8 cores gets 4 images and a
replicated copy of the weights; the host concatenates the 8 output shards.

Per-core kernel (v9): 1-D Winograd F(2,3) along H.
  The 3 kh-taps collapse into 4 m-plane matmul groups over stride-2 row
  tiles, cutting streamed PE columns 1.5x vs direct conv (12 matmuls of
  N=406 per 7-tile-row chunk instead of 18 equivalent direct columns).
  All values stay exact: x is host-binarized to +/-1 fp8; the Winograd
  weight transform G@g has entries in {+/-0.5, +/-1, +/-1.5} (exact in
  e4m3); RT = d_a +/- d_b in {-2,0,2}; products are multiples of 0.5
  accumulated in f32 PSUM; y = m0+m1+m2 / m1-m2-m3 are half-integer sums
  < 2048 so the fp16 output is exact. Bias is added on the host in f32.

  - host ships x pre-binarized AND pre-padded in the exact SBUF layout
    (one contiguous DMA per image, zero device-side memsets/binarize);
  - DVE computes the 4 RT input-transform planes (fp8 adds on strided
    row views); split per-chunk for pipelining;
  - per (image, co_blk, chunk of 7 tile rows): 12 DoubleRow fp8 matmuls
    (4 m-planes x 3 kw-taps) into 4 PSUM banks;
  - inverse transform fused into the drain: DVE does the even rows
    (m0+m1+m2), GpSimd the odd rows (m1-m2-m3), both PSUM->SBUF fp16;
  - y ships fp16 (half the writeback bytes), host upcasts + bias.
"""

import ml_dtypes
import numpy as np

import concourse.bacc as bacc
import concourse.mybir as mybir
import concourse.tile as tile
from concourse.bass_utils import run_bass_kernel_spmd

F32 = mybir.dt.float32
FP16 = mybir.dt.float16
BF16 = mybir.dt.bfloat16
FP8 = mybir.dt.float8e4
AF = mybir.ActivationFunctionType
ALU = mybir.AluOpType
DR = mybir.MatmulPerfMode.DoubleRow

N_CORES = 8
H = W = 56
WP = 58            # padded row width
CIN = 256
COUT = 256
CI_BLKS = 2
CO_BLKS = 2
FA = 3480          # per-ci_blk padded image elems (>= 2 + 58*a + 116*28)
NRT = 28           # F(2,3) tile rows (2 output rows each)
RQ = 7             # tile rows per chunk
NQ = NRT // RQ     # 4 chunks per (image, co_blk)
NV = RQ * WP       # 406 matmul moving free size
RTL = 1632         # per-(ci_blk, m) RT plane elems (2 guard + 28*58 + pad)
# RT_i = d_a + sign * d_b over padded rows (2*rt + a), i = 0..3
PAIRS = ((0, 2, ALU.subtract), (1, 2, ALU.add),
         (2, 1, ALU.subtract), (1, 3, ALU.subtract))
# image-0 x DMA pieces (in padded-image elems); piece p covers chunk p's rows
PIECES0 = ((0, 17 * WP), (17 * WP, 45 * WP), (45 * WP, FA))
JUNK_MM = 6


def _build_conv(tc, y_ap, x_ap, wt_ap, b_ap, n_imgs):
    nc = tc.nc

    with (
        tc.tile_pool(name="consts", bufs=1) as consts,
        tc.tile_pool(name="lhst", bufs=1) as lhst_pool,
        tc.tile_pool(name="xpad", bufs=1) as xpad_pool,
        tc.tile_pool(name="rt", bufs=1) as rt_pool,
        tc.tile_pool(name="s1", bufs=3) as s1_pool,
        tc.tile_pool(name="s3", bufs=3) as s3_pool,
        tc.tile_pool(name="tmpe", bufs=2) as tmpe_pool,
        tc.tile_pool(name="tmpo", bufs=2) as tmpo_pool,
        tc.tile_pool(name="outsb", bufs=2) as out_pool,
        tc.tile_pool(name="psum", bufs=8, space="PSUM") as psum_pool,
    ):
        junk = consts.tile([128, 512], BF16, name="junk")
        nc.gpsimd.memset(junk, 0.0)

        # Winograd weights: [128ci_p, 2ci_blk, 4m, 3kw, 256co] fp8
        lhst = lhst_pool.tile([128, CI_BLKS, 4, 3, COUT], FP8)
        # w DMA split per (m, ci_blk): first conv matmuls only need m=0
        for i in range(4):
            for cb in range(CI_BLKS):
                nc.scalar.dma_start(out=lhst[:, cb, i], in_=wt_ap[cb, i])
        bias_sb = consts.tile([128, CO_BLKS], F32)
        nc.scalar.dma_start(out=bias_sb,
                            in_=b_ap.rearrange("(b p) -> p b", p=128))

        xpads = [xpad_pool.tile([128, CI_BLKS, FA], FP8, name=f"xpad{n}")
                 for n in range(n_imgs)]
        NRTBUF = 3
        rts = [rt_pool.tile([128, CI_BLKS, 4, RTL], FP8, name=f"rt{j}")
               for j in range(NRTBUF)]
        for rtt in rts:
            # front guards (streamed but never drained) must be zero
            nc.vector.memset(rtt[:, 0:2, :, 0:2], 0.0)

        def dma_x(n, o0, o1):
            nc.sync.dma_start(
                out=xpads[n][:, 0:2, o0:o1],
                in_=x_ap[n].rearrange("b p f -> p b f")[:, :, o0:o1])

        def junk_mm():
            jps = psum_pool.tile([128, 512], F32, name="ps", tag="ps")
            nc.tensor.matmul(jps, junk[:, :128], junk, start=True, stop=True)

        def rt_piece(n, rtt, r0, r1):
            # RT_i tile-rows [r0, r1): reads padded rows 2r+a, writes
            # rt[:, :, i, 2 + 58*r0 : 2 + 58*r1]
            for i, (a, b, op) in enumerate(PAIRS):
                out = rtt[:, 0:2, i, 2 + WP * r0:2 + WP * r1].rearrange(
                    "p c (r w) -> p c r w", w=WP)

                def rows(aa):
                    off = 2 + WP * aa
                    return xpads[n][:, 0:2, off:off + 116 * NRT].rearrange(
                        "p c (r w2) -> p c r w2", w2=116)[:, :, r0:r1, 0:WP]

                nc.vector.tensor_tensor(out, rows(a), rows(b), op)

        def chunk(n, c, q, rtt, osb, per_chunk_y):
            pms = [psum_pool.tile([128, NV], F32, name="ps", tag="ps")
                   for _ in range(4)]
            for i in range(4):
                for kw in range(3):
                    base = WP * RQ * q + kw
                    nc.tensor.matmul(
                        pms[i],
                        lhst[:, 0:2, i, kw, c * 128:(c + 1) * 128],
                        rtt[:, 0:2, i, base:base + NV],
                        start=(kw == 0), stop=(kw == 2), perf_mode=DR)
            pmv = [p.rearrange("p (r w) -> p r w", w=WP)[:, :, 1:57]
                   for p in pms]
            yv = osb.rearrange("p (h2 t w) -> p h2 t w", t=2, w=W)
            ye = yv[:, RQ * q:RQ * (q + 1), 0, :]
            yo = yv[:, RQ * q:RQ * (q + 1), 1, :]
            # PSUM access rules: GpSimd none, DVE/Scalar one operand per op.
            # Scalar lifts m1(+bias) and m3 to SBUF; DVE does the PSUM math;
            # GpSimd finishes the odd rows SBUF-only.
            s1 = s1_pool.tile([128, RQ, W], F32, name="s1", tag="s1")
            nc.scalar.activation(out=s1, in_=pmv[1], func=AF.Identity,
                                 bias=bias_sb[:, c:c + 1], scale=1.0)
            s3 = s3_pool.tile([128, RQ, W], F32, name="s3", tag="s3")
            nc.scalar.activation(out=s3, in_=pmv[3], func=AF.Identity)
            tmp_e = tmpe_pool.tile([128, RQ, W], F32, name="te", tag="te")
            nc.vector.tensor_tensor(tmp_e, pmv[0], s1, ALU.add)
            nc.vector.tensor_tensor(ye, tmp_e, pmv[2], ALU.add)
            tmp_o = tmpo_pool.tile([128, RQ, W], F32, name="to", tag="to")
            nc.vector.tensor_tensor(tmp_o, s1, pmv[2], ALU.subtract)
            nc.gpsimd.tensor_tensor(yo, tmp_o, s3, ALU.subtract)
            if per_chunk_y:
                dma_y(n, c, osb, 2 * RQ * W * q, 2 * RQ * W * (q + 1))
            elif q == 1:
                dma_y(n, c, osb, 0, H * W // 2)

        def dma_y(n, c, osb, lo, hi):
            nc.scalar.dma_start(
                out=y_ap[n, c * 128:(c + 1) * 128, lo:hi],
                in_=osb[:, lo:hi])

        def coblk(n, c, rtt, interleave=None, per_chunk_y=False):
            osb = out_pool.tile([128, H * W], FP16, name="osb", tag="osb")
            for q in range(NQ):
                if interleave is not None:
                    interleave(q)
                chunk(n, c, q, rtt, osb, per_chunk_y)
            if not per_chunk_y:
                dma_y(n, c, osb, H * W // 2, H * W)

        # --- image 0 ramp: x pieces / transforms / chunks interleaved so the
        # dep tracker's byte-range coarsening can't chain early chunks onto
        # late pieces; junk matmuls keep the PE clock gate warm meanwhile.
        rt0 = rts[0]
        dma_x(0, *PIECES0[0])
        for _ in range(JUNK_MM):
            junk_mm()
        osb0 = out_pool.tile([128, H * W], FP16, name="osb", tag="osb")
        for q in range(NQ):
            rt_piece(0, rt0, RQ * q, RQ * (q + 1))
            chunk(0, 0, q, rt0, osb0, False)
            if q < len(PIECES0) - 1:
                dma_x(0, *PIECES0[q + 1])
        dma_y(0, 0, osb0, H * W // 2, H * W)

        # steady state: image n's c1 sweep interleaves image n+1's transform
        for n in range(n_imgs):
            rtt = rts[n % NRTBUF]
            if n > 0:
                coblk(n, 0, rtt)
            last = n == n_imgs - 1
            if not last:
                if n + 1 < n_imgs:
                    dma_x(n + 1, 0, FA)
                nxt = rts[(n + 1) % NRTBUF]
                coblk(n, 1, rtt,
                      interleave=lambda q: rt_piece(n + 1, nxt,
                                                    RQ * q, RQ * (q + 1)))
            else:
                coblk(n, 1, rtt, per_chunk_y=True)


_NC_CACHE = {}


def _get_nc(n_imgs):
    if n_imgs not in _NC_CACHE:
        nc = bacc.Bacc("TRN2", target_bir_lowering=False, debug=False)
        x_ap = nc.dram_tensor("x", [n_imgs, CI_BLKS, 128, FA], FP8,
                              kind="ExternalInput").ap()
        wt_ap = nc.dram_tensor("wt", [CI_BLKS, 4, 128, 3, COUT], FP8,
                               kind="ExternalInput").ap()
        b_ap = nc.dram_tensor("bias", [COUT], F32, kind="ExternalInput").ap()
        y_ap = nc.dram_tensor("y", [n_imgs, COUT, H * W], FP16,
                              kind="ExternalOutput").ap()
        with tile.TileContext(nc) as tc:
            _build_conv(tc, y_ap, x_ap, wt_ap, b_ap, n_imgs)
        nc.compile()
        _NC_CACHE[n_imgs] = nc
    return _NC_CACHE[n_imgs]


def make_in_maps(x, weight, bias):
    """Host-side shard prep: x sign-binarized to +/-1 fp8 in the padded
    SBUF layout [n, ci_blk, 128, FA]; weights 1-D Winograd-transformed
    (G @ g along kh) to fp8 in [ci_blk, m, 128ci, kw, co] layout."""
    n_imgs = x.shape[0] // N_CORES
    N = x.shape[0]
    xp = np.zeros((N, CI_BLKS, 128, FA), np.uint8)
    xr = np.asarray(x, dtype=np.float32).reshape(N, CI_BLKS, 128, H, W)
    # fp8e4m3 bytes: +1.0 = 0x38, -1.0 = 0xB8 (data (h,c) at 60 + 58h + c)
    xp[:, :, :, 60:60 + H * WP].reshape(N, CI_BLKS, 128, H, WP)[..., :W] = \
        np.where(xr >= 0, np.uint8(0x38), np.uint8(0xB8))
    xp8 = xp.view(ml_dtypes.float8_e4m3)

    g = np.where(np.asarray(weight, dtype=np.float32) >= 0,
                 np.float32(1), np.float32(-1))       # [co, ci, kh, kw]
    wt = np.stack([g[:, :, 0, :],
                   (g[:, :, 0, :] + g[:, :, 1, :] + g[:, :, 2, :]) * 0.5,
                   (g[:, :, 0, :] - g[:, :, 1, :] + g[:, :, 2, :]) * 0.5,
                   g[:, :, 2, :]])                    # [4m, co, ci, kw]
    wt = wt.transpose(2, 0, 3, 1).reshape(CI_BLKS, 128, 4, 3, COUT)
    wt8 = np.ascontiguousarray(wt.transpose(0, 2, 1, 3, 4)).astype(
        ml_dtypes.float8_e4m3)                        # [cib, m, ci, kw, co]
    b = np.ascontiguousarray(bias, dtype=np.float32)
    return [{"x": np.ascontiguousarray(xp8[i * n_imgs:(i + 1) * n_imgs]),
             "wt": wt8, "bias": b}
            for i in range(N_CORES)]


def kernel(x: np.ndarray, weight: np.ndarray, bias: np.ndarray) -> np.ndarray:
    assert x.shape[1:] == (CIN, H, W), x.shape
    assert x.shape[0] % N_CORES == 0, x.shape
    n_imgs = x.shape[0] // N_CORES
    nc = _get_nc(n_imgs)
    in_maps = make_in_maps(x, weight, bias)
    res = run_bass_kernel_spmd(nc, in_maps, core_ids=list(range(N_CORES)))
    y16 = np.concatenate([r["y"] for r in res.results], axis=0)
    return y16.astype(np.float32).reshape(x.shape[0], COUT, H, W)


# revision 14
# speedup vs baseline: 1.3523x; 1.3523x over previous
"""Binarized 3x3 conv (stride 1, pad 1) + bias on 8 Trainium2 NeuronCores.

Full problem: x[32,256,56,56] f32, weight[256,256,3,3] f32, bias[256] f32
-> y[32,256,56,56] f32 with y = conv2d(sign(x), sign(weight), pad=1) + bias
(sign(t) = +1 for t >= 0 else -1).

Sharding: data-parallel over batch. Each of the 8 cores gets 4 images and a
replicated copy of the weights; the host concatenates the 8 output shards.

Per-core kernel (v10): 1-D Winograd F(2,3) along H, host-side input transform.
  The 3 kh-taps collapse into 4 m-plane matmul groups over stride-2 row
  tiles, cutting streamed PE columns 1.5x vs direct conv (12 matmuls of
  N=406 per 7-tile-row chunk instead of 18 equivalent direct columns).
  Everything stays exact: the host binarizes x to +/-1 and ships the 4
  RT input-transform planes (d_a +/- d_b in {-2,0,2}) as fp8; the weight
  transform G@g has entries in {+/-0.5, +/-1, +/-1.5} (exact in e4m3);
  products are multiples of 0.5 accumulated in f32 PSUM; y rows are
  m0+m1+m2 / m1-m2-m3 -- half-integer sums well under 2048, so the fp16
  output rounds only the +bias term (~1e-4 rel err).

  - per (image, co_blk, chunk of 7 tile rows): 12 DoubleRow fp8 matmuls
    (4 m-planes x 3 kw-taps) into 4 PSUM banks (~2.03us/chunk, the pace);
  - drain/inverse-transform per chunk, split to fit engine budgets
    (PSUM readable only by Scalar/DVE, one operand per op):
      Scalar: s1 = m1 + bias, s3 = m3            (2 ACTs, 1.17us)
      DVE:    t = m0 + s1; ye = t + m2; u = s1 - m2   (3 ops, 1.66us)
      GpSimd: yo = u - s3                        (1 op, 1.19us)
  - y ships fp16 (halved writeback), host upcasts to f32.
"""

import ml_dtypes
import numpy as np

import concourse.bacc as bacc
import concourse.mybir as mybir
import concourse.tile as tile
from concourse.bass_utils import run_bass_kernel_spmd

F32 = mybir.dt.float32
FP16 = mybir.dt.float16
BF16 = mybir.dt.bfloat16
FP8 = mybir.dt.float8e4
AF = mybir.ActivationFunctionType
ALU = mybir.AluOpType
DR = mybir.MatmulPerfMode.DoubleRow

N_CORES = 8
H = W = 56
WP = 58            # padded row width
CIN = 256
COUT = 256
CI_BLKS = 2
CO_BLKS = 2
NRT = 28           # F(2,3) tile rows (2 output rows each)
RQ = 7             # tile rows per chunk
NQ = NRT // RQ     # 4 chunks per (image, co_blk)
NV = RQ * WP       # 406 matmul moving free size
RTL = 1632         # per-(ci_blk, m) RT plane elems (2 guard + 28*58 + pad)
JUNK_MM = 6


def _build_conv(tc, y_ap, rt_ap, wt_ap, b_ap, n_imgs):
    nc = tc.nc

    with (
        tc.tile_pool(name="consts", bufs=1) as consts,
        tc.tile_pool(name="lhst", bufs=1) as lhst_pool,
        tc.tile_pool(name="rt", bufs=1) as rt_pool,
        tc.tile_pool(name="s1", bufs=3) as s1_pool,
        tc.tile_pool(name="s3", bufs=3) as s3_pool,
        tc.tile_pool(name="tmpe", bufs=2) as tmpe_pool,
        tc.tile_pool(name="tmpo", bufs=2) as tmpo_pool,
        tc.tile_pool(name="outsb", bufs=2) as out_pool,
        tc.tile_pool(name="psum", bufs=8, space="PSUM") as psum_pool,
    ):
        junk = consts.tile([128, 512], BF16, name="junk")
        nc.gpsimd.memset(junk, 0.0)

        # Winograd weights: [128ci_p, 2ci_blk, 4m, 3kw, 256co] fp8
        lhst = lhst_pool.tile([128, CI_BLKS, 4, 3, COUT], FP8)
        # w DMA split per (m, ci_blk): first conv matmuls only need m=0
        for i in range(4):
            for cb in range(CI_BLKS):
                nc.scalar.dma_start(out=lhst[:, cb, i], in_=wt_ap[cb, i])
        bias_sb = consts.tile([128, CO_BLKS], F32)
        nc.scalar.dma_start(out=bias_sb,
                            in_=b_ap.rearrange("(b p) -> p b", p=128))

        NRTBUF = 3
        rts = [rt_pool.tile([128, CI_BLKS, 4, RTL], FP8, name=f"rt{j}")
               for j in range(NRTBUF)]

        def dma_rt(n, o0, o1):
            nc.sync.dma_start(
                out=rts[n % NRTBUF][:, 0:2, :, o0:o1],
                in_=rt_ap[n].rearrange("c i p f -> p c i f")[:, :, :, o0:o1])

        def junk_mm():
            jps = psum_pool.tile([128, 512], F32, name="ps", tag="ps")
            nc.tensor.matmul(jps, junk[:, :128], junk, start=True, stop=True)

        def chunk(n, c, q, rtt, osb, per_chunk_y):
            pms = [psum_pool.tile([128, NV], F32, name="ps", tag="ps")
                   for _ in range(4)]
            for i in range(4):
                for kw in range(3):
                    base = WP * RQ * q + kw
                    nc.tensor.matmul(
                        pms[i],
                        lhst[:, 0:2, i, kw, c * 128:(c + 1) * 128],
                        rtt[:, 0:2, i, base:base + NV],
                        start=(kw == 0), stop=(kw == 2), perf_mode=DR)
            pmv = [p.rearrange("p (r w) -> p r w", w=WP)[:, :, 1:57]
                   for p in pms]
            yv = osb.rearrange("p (h2 t w) -> p h2 t w", t=2, w=W)
            ye = yv[:, RQ * q:RQ * (q + 1), 0, :]
            yo = yv[:, RQ * q:RQ * (q + 1), 1, :]
            # PSUM access rules: GpSimd none, DVE/Scalar one operand per op.
            s1 = s1_pool.tile([128, RQ, W], F32, name="s1", tag="s1")
            nc.scalar.activation(out=s1, in_=pmv[1], func=AF.Identity,
                                 bias=bias_sb[:, c:c + 1], scale=1.0)
            s3 = s3_pool.tile([128, RQ, W], F32, name="s3", tag="s3")
            nc.scalar.activation(out=s3, in_=pmv[3], func=AF.Identity)
            tmp_e = tmpe_pool.tile([128, RQ, W], F32, name="te", tag="te")
            nc.vector.tensor_tensor(tmp_e, pmv[0], s1, ALU.add)
            nc.vector.tensor_tensor(ye, tmp_e, pmv[2], ALU.add)
            tmp_o = tmpo_pool.tile([128, RQ, W], F32, name="to", tag="to")
            nc.vector.tensor_tensor(tmp_o, s1, pmv[2], ALU.subtract)
            nc.gpsimd.tensor_tensor(yo, tmp_o, s3, ALU.subtract)
            if per_chunk_y:
                dma_y(n, c, osb, 2 * RQ * W * q, 2 * RQ * W * (q + 1))
            elif q == 1:
                dma_y(n, c, osb, 0, H * W // 2)

        def dma_y(n, c, osb, lo, hi):
            nc.scalar.dma_start(
                out=y_ap[n, c * 128:(c + 1) * 128, lo:hi],
                in_=osb[:, lo:hi])

        def coblk(n, c, rtt, per_chunk_y=False):
            osb = out_pool.tile([128, H * W], FP16, name="osb", tag="osb")
            for q in range(NQ):
                chunk(n, c, q, rtt, osb, per_chunk_y)
            if not per_chunk_y:
                dma_y(n, c, osb, H * W // 2, H * W)

        # --- image 0 ramp: rt arrives in 4 chunk-aligned pieces, each
        # chunk's matmuls emitted right after its piece so the dep tracker
        # can't chain early chunks onto late pieces; junk matmuls keep the
        # PE clock gate warm meanwhile. Pieces overlap 2 cols (chunk q
        # streams 2 cols into row 7(q+1)'s guard-zero region).
        dma_rt(0, 0, WP * RQ + 2)
        for _ in range(JUNK_MM):
            junk_mm()
        osb0 = out_pool.tile([128, H * W], FP16, name="osb", tag="osb")
        for q in range(NQ):
            chunk(0, 0, q, rts[0], osb0, False)
            if q < NQ - 1:
                dma_rt(0, WP * RQ * (q + 1), min(WP * RQ * (q + 2) + 2, RTL))
        dma_y(0, 0, osb0, H * W // 2, H * W)

        # image n+1's rt DMA (~10us) is issued ~2 sweeps (~16us) ahead
        if n_imgs > 1:
            dma_rt(1, 0, RTL)
        for n in range(n_imgs):
            rtt = rts[n % NRTBUF]
            if n > 0:
                if n + 1 < n_imgs:
                    dma_rt(n + 1, 0, RTL)
                coblk(n, 0, rtt)
            coblk(n, 1, rtt, per_chunk_y=(n == n_imgs - 1))


_NC_CACHE = {}


def _get_nc(n_imgs):
    if n_imgs not in _NC_CACHE:
        nc = bacc.Bacc("TRN2", target_bir_lowering=False, debug=False)
        rt_ap = nc.dram_tensor("x", [n_imgs, CI_BLKS, 4, 128, RTL], FP8,
                               kind="ExternalInput").ap()
        wt_ap = nc.dram_tensor("wt", [CI_BLKS, 4, 128, 3, COUT], FP8,
                               kind="ExternalInput").ap()
        b_ap = nc.dram_tensor("bias", [COUT], F32, kind="ExternalInput").ap()
        y_ap = nc.dram_tensor("y", [n_imgs, COUT, H * W], FP16,
                              kind="ExternalOutput").ap()
        with tile.TileContext(nc) as tc:
            _build_conv(tc, y_ap, rt_ap, wt_ap, b_ap, n_imgs)
        nc.compile()
        _NC_CACHE[n_imgs] = nc
    return _NC_CACHE[n_imgs]


# fp8e4m3 byte encodings for {-2, -1, 0, +1, +2} indexed by v + 2
# (boundary tile-rows mix pad zeros with +/-1 data, so odd values occur)
_FP8_LUT = np.array([0xC0, 0xB8, 0x00, 0x38, 0x40], dtype=np.uint8)


def make_in_maps(x, weight, bias):
    """Host-side shard prep: sign-binarize x and apply the F(2,3) input
    transform along H (RT_i = d_a +/- d_b over padded stride-2 rows),
    shipping 4 fp8 planes per (image, ci_blk) in the streaming layout
    [n, ci_blk, m, 128, RTL] (data (rtile, j) at col 2 + 58*rtile + j).
    Weights are 1-D Winograd-transformed (G @ g along kh) to fp8."""
    n_imgs = x.shape[0] // N_CORES
    N = x.shape[0]
    xs = np.where(np.asarray(x, dtype=np.float32) >= 0,
                  np.int8(1), np.int8(-1)).reshape(N, CI_BLKS, 128, H, W)
    # padded rows/cols: row r = x row r-1, cols 0..55 data, 56..57 zero
    # (the left-pad of a row is the previous row's col-57 zero in-stream)
    xp = np.zeros((N, CI_BLKS, 128, WP, WP), np.int8)
    xp[:, :, :, 1:57, :W] = xs
    rt = np.zeros((N, CI_BLKS, 4, 128, RTL), np.uint8)
    rtv = rt[:, :, :, :, 2:2 + NRT * WP].reshape(
        N, CI_BLKS, 4, 128, NRT, WP)
    for i, (a, b, s) in enumerate(((0, 2, -1), (1, 2, 1),
                                   (2, 1, -1), (1, 3, -1))):
        v = xp[:, :, :, a:a + 2 * NRT:2] + np.int8(s) * \
            xp[:, :, :, b:b + 2 * NRT:2]
        rtv[:, :, i] = _FP8_LUT[v.astype(np.int16) + 2]
    rt8 = rt.view(ml_dtypes.float8_e4m3)

    g = np.where(np.asarray(weight, dtype=np.float32) >= 0,
                 np.float32(1), np.float32(-1))       # [co, ci, kh, kw]
    wt = np.stack([g[:, :, 0, :],
                   (g[:, :, 0, :] + g[:, :, 1, :] + g[:, :, 2, :]) * 0.5,
                   (g[:, :, 0, :] - g[:, :, 1, :] + g[:, :, 2, :]) * 0.5,
                   g[:, :, 2, :]])                    # [4m, co, ci, kw]
    wt = wt.transpose(2, 0, 3, 1).reshape(CI_BLKS, 128, 4, 3, COUT)
    wt8 = np.ascontiguousarray(wt.transpose(0, 2, 1, 3, 4)).astype(
        ml_dtypes.float8_e4m3)                        # [cib, m, ci, kw, co]
    b = np.ascontiguousarray(bias, dtype=np.float32)
    return [{"x": np.ascontiguousarray(rt8[i * n_imgs:(i + 1) * n_imgs]),
             "wt": wt8, "bias": b}
            for i in range(N_CORES)]


def kernel(x: np.ndarray, weight: np.ndarray, bias: np.ndarray) -> np.ndarray:
    assert x.shape[1:] == (CIN, H, W), x.shape
    assert x.shape[0] % N_CORES == 0, x.shape
    n_imgs = x.shape[0] // N_CORES
    nc = _get_nc(n_imgs)
    in_maps = make_in_maps(x, weight, bias)
    res = run_bass_kernel_spmd(nc, in_maps, core_ids=list(range(N_CORES)))
    y16 = np.concatenate([r["y"] for r in res.results], axis=0)
    return y16.astype(np.float32).reshape(x.shape[0], COUT, H, W)
